# revision 1
# baseline (speedup 1.0000x reference)
"""XNOR-Net style binarized 3x3 conv (BinConv2d) on 8 Trainium2 NeuronCores.

Math: out = conv(sign(x)*mean|x|, sign(w)*mean|w|) + b
         = (mean|x| * mean|w|) * conv(sign(x), sign(w)) + b

The conv operands are pure {-1, 0, +1}, exactly representable in fp8e4m3, and
all partial sums are small integers held exactly in fp32 PSUM, so the heavy
conv runs on the tensor engine in fp8 with DoubleRow (2x) perf mode.

Layout (per core, data-parallel over batch: 4 images/core):
  - padded sign image per (image): flat fp8 SBUF buffer, 58 rows x 58 cols
    (pad=1 ring of zeros), flattened; conv = 9 shift-matmuls over C=128
    (partition/contraction dim), accumulated in PSUM.
  - the 9 kernel taps are grouped into 4 DoubleRow pairs + 1 plain matmul.
  - eviction fuses scale (mean|x|*mean|w|) and bias via ScalarE Identity.
"""

import numpy as np
import ml_dtypes

# Problem constants (hardcoded per contest rules)
N, C, H, W = 32, 128, 56, 56
K, KS = 256, 3
NCORES = 8
NPC = N // NCORES          # images per core
RS = H + 2                 # padded row stride = 58
PLANE = RS * RS            # 3364
IMG_LEN = 3712             # >= 59 + 6*512 + 511 + 59 + 1 = 3702, rounded up
S0 = RS + 1                # first interior flat index = 59
NTILE = 7                  # 7 tiles of 512 cover interior [59, 3304]
TS = 512                   # spatial tile (psum free dim)
OS_ROWS, OS_COLS = 62, RS  # output staging viewed [62, 58]; 62*58=3596>=7*512

# tap order: pairs (0,1),(2,3),(4,5),(6,7) are DoubleRow pairs; 8 is single
ORD = [(-1, -1), (-1, 0), (-1, 1), (0, -1), (0, 0), (0, 1), (1, -1), (1, 0), (1, 1)]
OFF = [dy * RS + dx for (dy, dx) in ORD]


def build_program(scale: float):
    from concourse import bass, bacc, tile, mybir

    FP8 = mybir.dt.float8e4
    F32 = mybir.dt.float32
    DR = mybir.MatmulPerfMode.DoubleRow
    ACT_ID = mybir.ActivationFunctionType.Identity
    ACT_SIGN = mybir.ActivationFunctionType.Sign

    nc = bacc.Bacc("TRN2", target_bir_lowering=False, debug=False)
    x_d = nc.dram_tensor("x", [NPC, C, H, W], F32, kind="ExternalInput").ap()
    wt_d = nc.dram_tensor("wt", [9, C, K], FP8, kind="ExternalInput").ap()
    b_d = nc.dram_tensor("b2", [C, K // C], F32, kind="ExternalInput").ap()
    out_d = nc.dram_tensor("out", [NPC, K, H, W], F32, kind="ExternalOutput").ap()

    with tile.TileContext(nc) as tc:
        with (
            tc.tile_pool(name="const", bufs=1) as const_p,
            tc.tile_pool(name="raw", bufs=2) as raw_p,
            tc.tile_pool(name="img", bufs=2) as img_p,
            tc.tile_pool(name="os", bufs=4) as os_p,
            tc.tile_pool(name="ps", bufs=4, space="PSUM") as ps_p,
        ):
            wt = const_p.tile([C, 9, K], FP8, tag="wt")
            nc.sync.dma_start(out=wt[:], in_=wt_d[:].transpose([1, 0, 2]))
            bias = const_p.tile([C, K // C], F32, tag="bias")
            nc.sync.dma_start(out=bias[:], in_=b_d[:])

            for i in range(NPC):
                # --- build padded sign image ---
                raw = raw_p.tile([C, H, W], F32, tag="raw")
                nc.sync.dma_start(out=raw[:], in_=x_d[i])
                img = img_p.tile([C, IMG_LEN], FP8, tag="img")
                iap = img[:]
                pdim = list(iap.ap[0])  # [partition_stride, 128]

                def iview(off, ap_dims):
                    return bass.AP(tensor=iap.tensor, offset=iap.offset + off,
                                   ap=[pdim] + ap_dims)

                # zero the pad ring (and trailing garbage-read region)
                nc.vector.memset(iview(0, [[1, S0]]), 0.0)                # row 0 + (1,0)
                nc.vector.memset(iview(2 * RS - 1, [[RS, 55], [1, 2]]), 0.0)  # (y,57),(y+1,0)
                nc.vector.memset(iview(PLANE - RS - 1, [[1, IMG_LEN - PLANE + RS + 1]]), 0.0)
                # interior = sign(x), written strided into the padded buffer
                nc.scalar.activation(iview(S0, [[RS, H], [1, W]]), raw[:], ACT_SIGN)

                # --- conv: 2 k-tiles x 7 spatial tiles, 5 matmuls each ---
                for kt in range(K // C):
                    os = os_p.tile([C, OS_ROWS, OS_COLS], F32, tag="os")
                    os_flat = os[:].rearrange("p a b -> p (a b)")
                    for t in range(NTILE):
                        s0 = S0 + TS * t
                        ps = ps_p.tile([C, TS], F32, tag="ps")
                        for p in range(4):
                            a, b = OFF[2 * p], OFF[2 * p + 1]
                            rhs = iview(s0 + a, [[b - a, 2], [1, TS]])
                            lhsT = wt[:, 2 * p:2 * p + 2, kt * C:(kt + 1) * C]
                            nc.tensor.matmul(ps[:], lhsT, rhs, start=(p == 0),
                                             stop=False, perf_mode=DR)
                        rhs1 = iview(s0 + OFF[8], [[1, TS]])
                        nc.tensor.matmul(ps[:], wt[:, 8, kt * C:(kt + 1) * C], rhs1,
                                         start=False, stop=True)
                        # out = psum * (mean|x|*mean|w|) + bias[k]
                        nc.scalar.activation(os_flat[:, TS * t:TS * (t + 1)], ps[:],
                                             ACT_ID, scale=float(scale),
                                             bias=bias[:, kt:kt + 1])
                    nc.sync.dma_start(out=out_d[i, kt * C:(kt + 1) * C],
                                      in_=os[:, 0:H, 0:W])
    nc.compile()
    return nc


def kernel(input: np.ndarray, weight: np.ndarray, bias: np.ndarray) -> np.ndarray:
    from concourse.bass_utils import run_bass_kernel_spmd

    x = np.ascontiguousarray(input, dtype=np.float32)
    w = np.asarray(weight, dtype=np.float32)
    b = np.asarray(bias, dtype=np.float32)

    # global binarization scalars (tiny, replicated)
    sx = float(np.abs(x, dtype=np.float32).mean(dtype=np.float64))
    sw = float(np.abs(w).mean(dtype=np.float64))
    scale = np.float32(sx * sw)

    # weights: sign, tap-major, transposed to [tap, C, K], fp8
    ws = np.sign(w)  # (K, C, 3, 3)
    wt = np.stack([ws[:, :, dy + 1, dx + 1].T for (dy, dx) in ORD])  # (9, C, K)
    wt = np.ascontiguousarray(wt).astype(ml_dtypes.float8_e4m3fn)
    b2 = np.ascontiguousarray(b.reshape(K // C, C).T)  # [C, 2]

    nc = build_program(scale)
    in_maps = [
        {"x": x[i * NPC:(i + 1) * NPC], "wt": wt, "b2": b2} for i in range(NCORES)
    ]
    res = run_bass_kernel_spmd(nc, in_maps, list(range(NCORES)))
    out = np.concatenate([res.results[i]["out"] for i in range(NCORES)], axis=0)
    return out.astype(np.float32)


if __name__ == "__main__":
    rng = np.random.default_rng(0)
    x = rng.normal(size=(N, C, H, W)).astype(np.float32)
    w = rng.normal(size=(K, C, KS, KS)).astype(np.float32)
    b = rng.normal(size=(K,)).astype(np.float32)
    o = kernel(input=x, weight=w, bias=b)
    print(o.shape, o.dtype)
